# revision 4
# baseline (speedup 1.0000x reference)
"""Attention-only Llama forward on 8 trn2 NeuronCores — v2.

Sharding: 2 batch groups x 4-core tensor-parallel head groups.
Core c handles batch b = c//4 and heads [4g:4g+4], g = c%4.

v2 restructures each layer into two half-token passes (h0 = tokens
0:512, h1 = 512:1024) so the o_proj all-reduce is split per half and
pipelined: AR(h0) runs while h1 computes, AR(h1) runs while the next
layer's h0 computes.  Collectives dispatch from the gpsimd queue only;
staging DMAs run on sync; weight loads on scalar.

Per-core dataflow (activations transposed: xT [D, T'] with D on
partitions as 8 tiles of 128, token halves in separate tiles):
  - embedding: device indirect-DMA gather over a host-deduplicated table,
    then PE transposes into the half tiles.
  - q/k projections emit even/odd RoPE components separately via
    host-permuted weight columns; RoPE is 6 partition-aligned DVE ops
    per tensor-half covering all 4 heads at once.
  - scores computed transposed (scoresT [k, q]) as K=32 matmul pairs per
    head; causal masking by skipping fully-masked blocks + one triangular
    mask tile on diagonal blocks.
  - softmax denominator via a ones-column appended to v (row 64 of the
    attention-output accumulator), broadcast across partitions with a K=1
    ones matmul.
  - o_proj partials all-reduced in bf16 across each 4-core head group,
    one collective per token half.
  - final RMSNorm in fp32 after PE-transposing back to [T, D].

Matmul inputs are bf16 (1 cycle/row on PE); all accumulation is fp32.
"""

import json
import math
import os
import sys

for _p in ("/opt/trn_rl_repo", "/root/.axon_site/_ro/trn_rl_repo"):
    if os.path.isdir(_p) and _p not in sys.path:
        sys.path.insert(0, _p)

import ml_dtypes
import numpy as np

import concourse.bass as bass
import concourse.tile as tile
from concourse import mybir
from concourse.bass import ds, ts
from concourse.bass_utils import run_bass_kernel_spmd
from concourse.masks import make_identity

L = 4
D = 1024
H = 16
HD = 64
V = 32000
B = 2
T = 1024
THETA = 10000.0
EPS = 1e-5

N_CORES = 8
GROUPS = [[0, 2, 4, 6], [1, 3, 5, 7]]
HPC = 4          # heads per core
C = HPC * HD     # head-col slice width per core = 256
TH = T // 2      # tokens per half

F32 = mybir.dt.float32
BF16 = mybir.dt.bfloat16
I32 = mybir.dt.int32
Exp = mybir.ActivationFunctionType.Exp
Square = mybir.ActivationFunctionType.Square
Sqrt = mybir.ActivationFunctionType.Sqrt


# --- walrus compat: this build supports at most one sync wait per
# instruction; split extras onto EventSemaphore insts on the same engine.
def _split_multi_waits(bir: dict) -> int:
    ctr = 0
    for f in bir.get("functions", []):
        for bb in f.get("blocks", []):
            insts = bb.get("instructions", [])
            if not any(
                len((i.get("sync_info") or {}).get("on_wait") or []) > 1
                for i in insts
            ):
                continue
            out = []
            for inst in insts:
                si = inst.get("sync_info")
                waits = (si or {}).get("on_wait") or []
                if len(waits) > 1:
                    for w in waits[:-1]:
                        ctr += 1
                        out.append({
                            "debug": inst.get("debug"),
                            "engine": inst["engine"],
                            "ins": [],
                            "name": f"WSPLIT-{ctr}",
                            "opcode": "EventSemaphore",
                            "outs": [],
                            "sync_info": {"on_update": [], "on_wait": [w]},
                        })
                    si["on_wait"] = [waits[-1]]
                out.append(inst)
            bb["instructions"] = out
    return ctr


class CompatBass(bass.Bass):
    def to_json_bytes(self) -> bytes:
        raw = super().to_json_bytes()
        bir = json.loads(raw)
        if _split_multi_waits(bir):
            return json.dumps(bir).encode()
        return raw


def _build_program():
    nc = CompatBass(num_devices=N_CORES)

    # packed inputs: fewer PJRT buffers -> lower per-call dispatch cost.
    # bpack (bf16): per layer [wqe|wqo|wke|wko|wv|wo] then the embed table.
    # fpack (f32):  [cosq|sinq|cosk|sink|trimask|normw].
    PLW = 8 * 128 * 1024            # per-layer bf16 weight block
    BP_EMB = 0                      # embed table first (gather needs offset 0)
    BP_W = T * D                    # weights after the table
    bpack_ext = nc.declare_dram_parameter("bpack", [L * PLW + T * D], BF16,
                                          isOutput=False)
    fpack_ext = nc.declare_dram_parameter("fpack", [4 * 128 * T + 128 * 128 + D],
                                          F32, isOutput=False)
    out_ext = nc.declare_dram_parameter("out", [256, D], F32, isOutput=True)

    def bview(off, shape):
        ap, s = [], 1
        for n in reversed(shape):
            ap.insert(0, [s, n])
            s *= n
        base = bpack_ext[:]
        return bass.AP(tensor=base.tensor, offset=off, ap=ap)

    def fview(off, shape):
        ap, s = [], 1
        for n in reversed(shape):
            ap.insert(0, [s, n])
            s *= n
        base = fpack_ext[:]
        return bass.AP(tensor=base.tensor, offset=off, ap=ap)

    WQE_O = BP_W
    WQO_O = BP_W + 128 * 1024
    WKE_O = BP_W + 2 * 128 * 1024
    WKO_O = BP_W + 3 * 128 * 1024
    WV_O = BP_W + 4 * 128 * 1024
    WO_O = BP_W + 4 * 128 * 1024 + 128 * 8 * 256
    emb_ext = bview(BP_EMB, [T, D])
    cosq_ext = fview(0, [128, T])
    sinq_ext = fview(128 * T, [128, T])
    cosk_ext = fview(2 * 128 * T, [128, T])
    sink_ext = fview(3 * 128 * T, [128, T])
    mask_ext = fview(4 * 128 * T, [128, 128])
    normw_off = 4 * 128 * T + 128 * 128

    from contextlib import ExitStack

    with tile.TileContext(nc) as tc, ExitStack() as stack:
        ec = stack.enter_context
        cpool = ec(tc.tile_pool(name="consts", bufs=1))
        xpool = ec(tc.tile_pool(name="xT", bufs=2))
        wpool = ec(tc.tile_pool(name="weights", bufs=2))
        qkpool = ec(tc.tile_pool(name="qk", bufs=1))
        rotpool = ec(tc.tile_pool(name="rot", bufs=2))
        tmppool = ec(tc.tile_pool(name="ropetmp", bufs=1))
        vpool = ec(tc.tile_pool(name="v", bufs=1))
        atpool = ec(tc.tile_pool(name="att", bufs=3))
        otpool = ec(tc.tile_pool(name="oT", bufs=2))
        xppool = ec(tc.tile_pool(name="xpart", bufs=1))
        gpool = ec(tc.tile_pool(name="gather", bufs=1))
        npool = ec(tc.tile_pool(name="norm", bufs=1))
        mpool = ec(tc.tile_pool(name="misc", bufs=1))
        dpool = ec(tc.tile_pool(name="dram", bufs=2, space="DRAM"))
        pspool = ec(tc.tile_pool(name="ps_mm", bufs=2, space="PSUM"))
        scpool = ec(tc.tile_pool(name="ps_sc", bufs=2, space="PSUM"))
        oaccpool = ec(tc.tile_pool(name="ps_oacc", bufs=1, space="PSUM"))

        identb = cpool.tile([128, 128], BF16, tag="identb")
        make_identity(nc, identb)
        trimask = cpool.tile([128, 128], F32, tag="trimask")
        nc.scalar.dma_start(out=trimask[:], in_=mask_ext)
        cosq = cpool.tile([128, T], F32, tag="cosq")
        nc.scalar.dma_start(out=cosq[:], in_=cosq_ext)
        sinq = cpool.tile([128, T], F32, tag="sinq")
        nc.scalar.dma_start(out=sinq[:], in_=sinq_ext)
        cosk = cpool.tile([128, T], F32, tag="cosk")
        nc.scalar.dma_start(out=cosk[:], in_=cosk_ext)
        sink = cpool.tile([128, T], F32, tag="sink")
        nc.scalar.dma_start(out=sink[:], in_=sink_ext)
        normw_b = cpool.tile([128, D], F32, tag="normw")
        _f = fpack_ext[:]
        nc.scalar.dma_start(
            out=normw_b[:],
            in_=bass.AP(tensor=_f.tensor, offset=normw_off,
                        ap=[[0, 128], [1, D]]),
        )
        ones64 = cpool.tile([1, 64], BF16, tag="ones64")
        nc.vector.memset(ones64[:], 1.0)
        epst = cpool.tile([128, 1], F32, tag="epst")
        nc.vector.memset(epst[:], EPS)

        def embed_half(h):
            """Load the host-pretransposed xT half tile (one DMA)."""
            xh = xpool.tile([128, 8, TH], BF16, tag=f"xh{h}", name=f"xh{h}")
            nc.sync.dma_start(
                out=xh[:], in_=bview(BP_EMB + h * 128 * 8 * TH, [128, 8, TH]))
            return xh

        xh = [embed_half(0), embed_half(1)]
        xh_next = [None, None]

        for l in range(L):
            # ---- weight loads (scalar queue; wpool bufs=2 prefetches)
            wqe = wpool.tile([128, 8, 128], BF16, tag="wqe")
            nc.scalar.dma_start(out=wqe[:], in_=bview(l * PLW + WQE_O, [128, 8, 128]))
            wqo = wpool.tile([128, 8, 128], BF16, tag="wqo")
            nc.scalar.dma_start(out=wqo[:], in_=bview(l * PLW + WQO_O, [128, 8, 128]))
            wke = wpool.tile([128, 8, 128], BF16, tag="wke")
            nc.scalar.dma_start(out=wke[:], in_=bview(l * PLW + WKE_O, [128, 8, 128]))
            wko = wpool.tile([128, 8, 128], BF16, tag="wko")
            nc.scalar.dma_start(out=wko[:], in_=bview(l * PLW + WKO_O, [128, 8, 128]))
            wv = wpool.tile([128, 8, 256], BF16, tag="wv")
            nc.scalar.dma_start(out=wv[:], in_=bview(l * PLW + WV_O, [128, 8, 256]))
            wo = []
            for g2 in range(2):
                wog = wpool.tile([128, D], BF16, tag=f"wo{g2}")
                nc.scalar.dma_start(
                    out=wog[:], in_=bview(l * PLW + WO_O + g2 * 128 * D, [128, D]))
                wo.append(wog)

            # rotated q/k tiles persist across both halves of the layer
            rot = {}
            for name in ("qe", "qo", "ke", "ko"):
                rot[name] = rotpool.tile([128, T], BF16, tag=f"r{name}",
                                         name=f"r{name}")
            r3 = {}
            for name in ("qe", "qo", "ke", "ko"):
                r3[name] = rotpool.tile([32, T], BF16, tag=f"r3{name}",
                                        name=f"r3{name}")
            vt = []
            for j in range(8):
                vj = vpool.tile([128, 4, 65], BF16, tag=f"v{j}", name=f"v{j}")
                vt.append(vj)
            oT = []
            for g2 in range(2):
                oTg = otpool.tile([128, T], BF16, tag=f"oT{g2}",
                                  name=f"oT{g2}")
                oT.append(oTg)
            hsl = []
            for h in range(4):
                if h < 3:
                    sl = slice(32 * h, 32 * h + 32)
                    hsl.append((rot["qe"][sl, :], rot["qo"][sl, :],
                                rot["ke"][sl, :], rot["ko"][sl, :]))
                else:
                    hsl.append((r3["qe"][:], r3["qo"][:],
                                r3["ke"][:], r3["ko"][:]))

            for half in range(2):
                hsl_ = ds(TH * half, TH)
                if l > 0:
                    xh[half] = xh_next[half]

                # ---- q/k projections for this half (even/odd components)
                comps = {}
                for name, wt in (("qe", wqe), ("qo", wqo),
                                 ("ke", wke), ("ko", wko)):
                    dst = qkpool.tile([128, TH], BF16, tag=f"c{name}")
                    ps = pspool.tile([128, TH], F32, tag="mm")
                    for k in range(8):
                        nc.tensor.matmul(
                            ps[:], lhsT=wt[:, k, :], rhs=xh[half][:, k, :],
                            start=(k == 0), stop=(k == 7),
                        )
                    nc.scalar.copy(out=dst[:], in_=ps[:])
                    comps[name] = dst

                # ---- RoPE for this half; q on DVE, k on Pool-free DVE mix
                for pre, cs, sn in (("q", cosq, sinq), ("k", cosk, sink)):
                    e_in, o_in = comps[pre + "e"], comps[pre + "o"]
                    ta = tmppool.tile([128, TH], F32, tag="tmpa")
                    tb = tmppool.tile([128, TH], F32, tag="tmpb")
                    nc.vector.tensor_mul(ta[:], e_in[:], cs[:, hsl_])
                    nc.vector.tensor_mul(tb[:], o_in[:], sn[:, hsl_])
                    nc.vector.tensor_tensor(
                        out=rot[pre + "e"][:, hsl_], in0=ta[:], in1=tb[:],
                        op=mybir.AluOpType.subtract)
                    tc2 = tmppool.tile([128, TH], F32, tag="tmpa")
                    td = tmppool.tile([128, TH], F32, tag="tmpb")
                    nc.vector.tensor_mul(tc2[:], e_in[:], sn[:, hsl_])
                    nc.vector.tensor_mul(td[:], o_in[:], cs[:, hsl_])
                    nc.vector.tensor_tensor(
                        out=rot[pre + "o"][:, hsl_], in0=tc2[:], in1=td[:],
                        op=mybir.AluOpType.add)

                # head 3 lives at partition base 96 (not addressable by the
                # PE); copy its 32 rows to base-0 tiles.
                for name in ("qe", "qo", "ke", "ko"):
                    nc.vector.tensor_copy(out=r3[name][:, hsl_],
                                          in_=rot[name][96:128, hsl_])

                # ---- v projection for this half, [t, c] layout, +ones col
                for jj in range(4):
                    j = 4 * half + jj
                    ps = pspool.tile([128, 256], F32, tag="mm")
                    for k in range(8):
                        nc.tensor.matmul(
                            ps[:], lhsT=xh[half][:, k, ts(jj, 128)],
                            rhs=wv[:, k, :],
                            start=(k == 0), stop=(k == 7),
                        )
                    nc.vector.memset(vt[j][:, :, 64:65], 1.0)
                    nc.vector.tensor_copy(
                        out=vt[j][:, :, 0:64],
                        in_=ps[:].rearrange("p (h d) -> p h d", h=4))

                # ---- attention for query half `half`
                qc = half
                oaccs = []
                for h in range(4):
                    oacch = oaccpool.tile([65, TH], F32, tag=f"oacc{h}",
                                          name=f"oacc{h}")
                    oaccs.append(oacch)
                jbmax = 3 if qc == 0 else 7
                for jb in range(jbmax + 1):
                    q_lo = max(TH * qc, 128 * jb)
                    n = TH * (qc + 1) - q_lo
                    ats = []
                    for h in range(4):
                        lqe, lqo, lke, lko = hsl[h]
                        sc = scpool.tile([128, TH], F32, tag="sc")
                        nc.tensor.matmul(
                            sc[:, :n], lhsT=lke[:, ts(jb, 128)],
                            rhs=lqe[:, ds(q_lo, n)], start=True, stop=False)
                        nc.tensor.matmul(
                            sc[:, :n], lhsT=lko[:, ts(jb, 128)],
                            rhs=lqo[:, ds(q_lo, n)], start=False, stop=True)
                        if q_lo == 128 * jb:
                            nc.vector.tensor_add(
                                out=sc[:, 0:128], in0=sc[:, 0:128],
                                in1=trimask[:])
                        at = atpool.tile([128, TH], BF16, tag="att")
                        nc.scalar.activation(out=at[:, :n], in_=sc[:, :n],
                                             func=Exp)
                        ats.append(at)
                    for h in range(4):
                        nc.tensor.matmul(
                            oaccs[h][:, ds(q_lo - TH * qc, n)],
                            lhsT=vt[jb][:, h, :], rhs=ats[h][:, :n],
                            start=(jb == 0), stop=(jb == jbmax))
                for h in range(4):
                    recip = mpool.tile([1, TH], F32, tag="recip")
                    nc.vector.reciprocal(out=recip[:], in_=oaccs[h][64:65, :])
                    recipb = mpool.tile([1, TH], BF16, tag="recipb")
                    nc.vector.tensor_copy(out=recipb[:], in_=recip[:])
                    bc_ps = scpool.tile([64, TH], F32, tag="sc")
                    nc.tensor.matmul(
                        bc_ps[:], lhsT=ones64[:], rhs=recipb[:],
                        start=True, stop=True)
                    bc_sb = mpool.tile([64, TH], F32, tag="bcsb")
                    nc.scalar.copy(out=bc_sb[:], in_=bc_ps[:])
                    if h % 2 == 0:
                        nc.vector.tensor_mul(
                            oT[h // 2][0:64, ds(TH * qc, TH)],
                            oaccs[h][0:64, :], bc_sb[:])
                    else:
                        # engine ops cannot write at a nonzero partition
                        # base on this toolchain; place via DMA instead
                        om = mpool.tile([64, TH], BF16, tag="om")
                        nc.vector.tensor_mul(om[:], oaccs[h][0:64, :],
                                             bc_sb[:])
                        nc.scalar.dma_start(
                            out=oT[h // 2][64:128, ds(TH * qc, TH)],
                            in_=om[:])

                # ---- o_proj partials for this half
                xpart = xppool.tile([128, 8, TH], BF16, tag=f"xp{half}",
                                    name=f"xp{half}")
                for e in range(8):
                    ps = pspool.tile([128, TH], F32, tag="mm")
                    for g2 in range(2):
                        nc.tensor.matmul(
                            ps[:], lhsT=wo[g2][:, ts(e, 128)],
                            rhs=oT[g2][:, ds(TH * qc, TH)],
                            start=(g2 == 0), stop=(g2 == 1))
                    nc.scalar.copy(out=xpart[:, e, :], in_=ps[:])

                if l < L - 1:
                    # ---- pipelined per-half all-reduce
                    arin = dpool.tile([128, 8, TH], BF16, tag=f"arin{half}")
                    arout = dpool.tile([128, 8, TH], BF16, tag=f"arout{half}")
                    nc.sync.dma_start(out=arin[:], in_=xpart[:])
                    nc.gpsimd.collective_compute(
                        "AllReduce", mybir.AluOpType.add,
                        ins=[arin[:]], outs=[arout[:]], replica_groups=GROUPS)
                    xh_next[half] = xpool.tile([128, 8, TH], BF16,
                                               tag=f"xh{half}",
                                               name=f"xh{half}n")
                    nc.sync.dma_start(out=xh_next[half], in_=arout[:])
                else:
                    # ---- last layer: reduce-scatter; rank r of each group
                    # receives the summed x for token chunk r.
                    if half == 0:
                        rsin = dpool.tile([4, 128, 8, 256], BF16, tag="rsin")
                        rsout = dpool.tile([128, 8, 256], BF16, tag="rsout")
                    for jch in range(2):
                        nc.sync.dma_start(
                            out=rsin[2 * half + jch],
                            in_=xpart[:, :, ds(256 * jch, 256)])
                    if half == 1:
                        nc.gpsimd.collective_compute(
                            "ReduceScatter", mybir.AluOpType.add,
                            ins=[rsin[:]], outs=[rsout[:]],
                            replica_groups=GROUPS)
                        xfin = xpool.tile([128, 8, 256], BF16, tag="xfin")
                        nc.sync.dma_start(out=xfin[:], in_=rsout[:])

        # ---- final RMSNorm (fp32) on this core's 256-token chunk
        for j in range(2):
            xrow = npool.tile([128, D], F32, tag="xrow")
            for k in range(8):
                tp = scpool.tile([128, 128], BF16, tag="sc")
                nc.tensor.transpose(tp[:], xfin[:, k, ts(j, 128)], identb[:])
                nc.vector.tensor_copy(out=xrow[:, ts(k, 128)], in_=tp[:])
            onorm = npool.tile([128, D], F32, tag="onorm")
            ssq = npool.tile([128, 1], F32, tag="ssq")
            nc.scalar.activation(out=onorm[:], in_=xrow[:], func=Square,
                                 accum_out=ssq[:])
            std = npool.tile([128, 1], F32, tag="std")
            nc.scalar.activation(out=std[:], in_=ssq[:], func=Sqrt,
                                 scale=1.0 / D, bias=epst[:, :1])
            rstd = npool.tile([128, 1], F32, tag="rstd")
            nc.vector.reciprocal(out=rstd[:], in_=std[:])
            nc.vector.tensor_scalar_mul(out=xrow[:], in0=xrow[:],
                                        scalar1=rstd[:, :1])
            nc.vector.tensor_mul(onorm[:], xrow[:], normw_b[:])
            nc.sync.dma_start(out=out_ext[ts(j, 128), :], in_=onorm[:])

    return nc


def _prep_inputs(toks, embed, Wq, Wk, Wv, Wo, norm_w):
    """Build the 8 per-core input maps from the full model inputs."""
    toks = np.asarray(toks)
    embed = np.asarray(embed, dtype=np.float32)
    Wq = np.asarray(Wq, dtype=np.float32)
    Wk = np.asarray(Wk, dtype=np.float32)
    Wv = np.asarray(Wv, dtype=np.float32)
    Wo = np.asarray(Wo, dtype=np.float32)
    norm_w = np.asarray(norm_w, dtype=np.float32)

    inv = 1.0 / (THETA ** (np.arange(0, HD, 2, dtype=np.float32) / HD))  # [32]
    ang = inv[:, None] * np.arange(T, dtype=np.float32)[None, :]         # [32, T]
    cos = np.cos(ang)
    sin = np.sin(ang)
    scale = 1.0 / math.sqrt(HD)
    cosq = np.tile(cos * scale, (4, 1)).astype(np.float32)
    sinq = np.tile(sin * scale, (4, 1)).astype(np.float32)
    cosk = np.tile(cos, (4, 1)).astype(np.float32)
    sink = np.tile(sin, (4, 1)).astype(np.float32)

    jj = np.arange(128)
    trimask = np.where(jj[:, None] <= jj[None, :], 0.0, -1e9).astype(np.float32)

    in_maps = []
    batch_tables = []
    for b in range(B):
        E = embed[np.asarray(toks[b], dtype=np.int64)].astype(
            ml_dtypes.bfloat16)                         # [T, D]
        xT = np.ascontiguousarray(
            E.reshape(2, T // 2, 8, 128).transpose(0, 3, 2, 1))
        batch_tables.append(xT)

    fpack = np.concatenate([cosq.ravel(), sinq.ravel(), cosk.ravel(),
                            sink.ravel(), trimask.ravel(), norm_w.ravel()])

    for c in range(N_CORES):
        b, g = c % 2, c // 2
        xT = batch_tables[b]
        heads = [4 * g + h for h in range(4)]
        ecols = np.concatenate([64 * ah + np.arange(0, 64, 2) for ah in heads])
        ocols = np.concatenate([64 * ah + np.arange(1, 64, 2) for ah in heads])
        vcols = np.arange(256 * g, 256 * g + 256)

        def tile_w(w):  # [L, D, 128 or 256] -> [L, 128, 8, n]
            n = w.shape[-1]
            return np.ascontiguousarray(
                w.reshape(L, 8, 128, n).transpose(0, 2, 1, 3)
            ).astype(ml_dtypes.bfloat16)

        wqe = tile_w(Wq[:, :, ecols]).reshape(L, -1)
        wqo = tile_w(Wq[:, :, ocols]).reshape(L, -1)
        wke = tile_w(Wk[:, :, ecols]).reshape(L, -1)
        wko = tile_w(Wk[:, :, ocols]).reshape(L, -1)
        wv = tile_w(Wv[:, :, vcols]).reshape(L, -1)
        wo = np.ascontiguousarray(
            Wo[:, vcols, :].reshape(L, 2, 128, D)).astype(
                ml_dtypes.bfloat16).reshape(L, -1)
        per_layer = np.concatenate([wqe, wqo, wke, wko, wv, wo], axis=1)
        bpack = np.concatenate([xT.ravel(), per_layer.ravel()])

        in_maps.append({
            "bpack": bpack,
            "fpack": fpack,
        })
    return in_maps


class _Runner:
    """Compile the SPMD program once; re-executable on the 8 cores.

    Mirrors concourse.bass2jax.run_bass_via_pjrt but caches an AOT
    fast-dispatch compile so repeated calls use the C++ dispatch path.
    """

    def __init__(self):
        import jax
        from jax.sharding import Mesh, NamedSharding, PartitionSpec

        try:
            from jax.experimental.shard_map import shard_map
        except ImportError:
            from jax.shard_map import shard_map

        from concourse import bass2jax

        bass2jax.install_neuronx_cc_hook()
        nc = _build_program()
        self._jax = jax

        partition_name = (
            nc.partition_id_tensor.name if nc.partition_id_tensor else None
        )
        in_names, out_names, out_avals, zero_outs = [], [], [], []
        for alloc in nc.m.functions[0].allocations:
            if not isinstance(alloc, mybir.MemoryLocationSet):
                continue
            name = alloc.memorylocations[0].name
            if alloc.kind == "ExternalInput":
                if name != partition_name:
                    in_names.append(name)
            elif alloc.kind == "ExternalOutput":
                out_names.append(name)
                shape = tuple(alloc.tensor_shape)
                dtype = mybir.dt.np(alloc.dtype)
                out_avals.append(jax.core.ShapedArray(shape, dtype))
                zero_outs.append(np.zeros(shape, dtype))
        self.in_names = list(in_names)
        self.out_names = out_names
        n_params = len(in_names)
        all_in_names = in_names + out_names
        if partition_name is not None:
            all_in_names = all_in_names + [partition_name]

        def _body(*args):
            operands = list(args)
            if partition_name is not None:
                operands.append(bass2jax.partition_id_tensor())
            outs = bass2jax._bass_exec_p.bind(
                *operands,
                out_avals=tuple(out_avals),
                in_names=tuple(all_in_names),
                out_names=tuple(out_names),
                lowering_input_output_aliases=(),
                sim_require_finite=True,
                sim_require_nnan=True,
                nc=nc,
            )
            return tuple(outs)

        devices = jax.devices()[:N_CORES]
        mesh = Mesh(np.asarray(devices), ("core",))
        in_specs = (PartitionSpec("core"),) * (n_params + len(out_names))
        out_specs = (PartitionSpec("core"),) * len(out_names)

        sh = NamedSharding(mesh, PartitionSpec("core"))
        in_structs = []
        for alloc in nc.m.functions[0].allocations:
            if not isinstance(alloc, mybir.MemoryLocationSet):
                continue
            name = alloc.memorylocations[0].name
            if alloc.kind == "ExternalInput" and name != partition_name:
                shape = tuple(alloc.tensor_shape)
                dtype = mybir.dt.np(alloc.dtype)
                in_structs.append(jax.ShapeDtypeStruct(
                    (N_CORES * shape[0], *shape[1:]), dtype, sharding=sh))
        for aval in out_avals:
            in_structs.append(jax.ShapeDtypeStruct(
                (N_CORES * aval.shape[0], *aval.shape[1:]), aval.dtype,
                sharding=sh))

        def _compile():
            return jax.jit(
                shard_map(_body, mesh=mesh, in_specs=in_specs,
                          out_specs=out_specs, check_rep=False),
                keep_unused=True,
            ).lower(*in_structs).compile()

        self._fn = bass2jax.fast_dispatch_compile(_compile)
        self._zero_outs = zero_outs
        self._out_avals = out_avals
        self._mesh = mesh
        self._pspec = PartitionSpec("core")

    def place(self, in_maps, on_device=False):
        cat = [
            np.concatenate([np.asarray(in_maps[c][n]) for c in range(N_CORES)],
                           axis=0)
            for n in self.in_names
        ]
        cat += [
            np.zeros((N_CORES * z.shape[0], *z.shape[1:]), z.dtype)
            for z in self._zero_outs
        ]
        if on_device:
            from jax.sharding import NamedSharding

            sh = NamedSharding(self._mesh, self._pspec)
            cat = [self._jax.device_put(a, sh) for a in cat]
        return cat

    def execute(self, placed):
        return self._fn(*placed)

    def run(self, in_maps):
        out_arrs = self.execute(self.place(in_maps))
        return [
            {
                n: np.asarray(out_arrs[i]).reshape(
                    N_CORES, *self._out_avals[i].shape)[c]
                for i, n in enumerate(self.out_names)
            }
            for c in range(N_CORES)
        ]


_CACHE = {}


def get_runner():
    if "runner" not in _CACHE:
        _CACHE["runner"] = _Runner()
    return _CACHE["runner"]


def kernel(toks, embed, Wq, Wk, Wv, Wo, norm_w):
    in_maps = _prep_inputs(toks, embed, Wq, Wk, Wv, Wo, norm_w)
    results = get_runner().run(in_maps)
    out = np.empty((B, T, D), dtype=np.float32)
    for c in range(N_CORES):
        b, g = c % 2, c // 2
        out[b, 256 * g: 256 * (g + 1)] = results[c]["out"]
    return out


# revision 5
# speedup vs baseline: 1.0747x; 1.0747x over previous
"""Attention-only Llama forward on 8 trn2 NeuronCores — v2.

Sharding: 2 batch groups x 4-core tensor-parallel head groups.
Core c handles batch b = c//4 and heads [4g:4g+4], g = c%4.

v2 restructures each layer into two half-token passes (h0 = tokens
0:512, h1 = 512:1024) so the o_proj all-reduce is split per half and
pipelined: AR(h0) runs while h1 computes, AR(h1) runs while the next
layer's h0 computes.  Collectives dispatch from the gpsimd queue only;
staging DMAs run on sync; weight loads on scalar.

Per-core dataflow (activations transposed: xT [D, T'] with D on
partitions as 8 tiles of 128, token halves in separate tiles):
  - embedding: device indirect-DMA gather over a host-deduplicated table,
    then PE transposes into the half tiles.
  - q/k projections emit even/odd RoPE components separately via
    host-permuted weight columns; RoPE is 6 partition-aligned DVE ops
    per tensor-half covering all 4 heads at once.
  - scores computed transposed (scoresT [k, q]) as K=32 matmul pairs per
    head; causal masking by skipping fully-masked blocks + one triangular
    mask tile on diagonal blocks.
  - softmax denominator via a ones-column appended to v (row 64 of the
    attention-output accumulator), broadcast across partitions with a K=1
    ones matmul.
  - o_proj partials all-reduced in bf16 across each 4-core head group,
    one collective per token half.
  - final RMSNorm in fp32 after PE-transposing back to [T, D].

Matmul inputs are bf16 (1 cycle/row on PE); all accumulation is fp32.
"""

import json
import math
import os
import sys

for _p in ("/opt/trn_rl_repo", "/root/.axon_site/_ro/trn_rl_repo"):
    if os.path.isdir(_p) and _p not in sys.path:
        sys.path.insert(0, _p)

import ml_dtypes
import numpy as np

import concourse.bass as bass
import concourse.tile as tile
from concourse import mybir
from concourse.bass import ds, ts
from concourse.bass_utils import run_bass_kernel_spmd
from concourse.masks import make_identity

L = 4
D = 1024
H = 16
HD = 64
V = 32000
B = 2
T = 1024
THETA = 10000.0
EPS = 1e-5

N_CORES = 8
GROUPS = [[0, 1, 2, 3], [4, 5, 6, 7]]
HPC = 4          # heads per core
C = HPC * HD     # head-col slice width per core = 256
TH = T // 2      # tokens per half

F32 = mybir.dt.float32
BF16 = mybir.dt.bfloat16
I32 = mybir.dt.int32
Exp = mybir.ActivationFunctionType.Exp
Square = mybir.ActivationFunctionType.Square
Sqrt = mybir.ActivationFunctionType.Sqrt


# --- walrus compat: this build supports at most one sync wait per
# instruction; split extras onto EventSemaphore insts on the same engine.
def _split_multi_waits(bir: dict) -> int:
    ctr = 0
    for f in bir.get("functions", []):
        for bb in f.get("blocks", []):
            insts = bb.get("instructions", [])
            if not any(
                len((i.get("sync_info") or {}).get("on_wait") or []) > 1
                for i in insts
            ):
                continue
            out = []
            for inst in insts:
                si = inst.get("sync_info")
                waits = (si or {}).get("on_wait") or []
                if len(waits) > 1:
                    for w in waits[:-1]:
                        ctr += 1
                        out.append({
                            "debug": inst.get("debug"),
                            "engine": inst["engine"],
                            "ins": [],
                            "name": f"WSPLIT-{ctr}",
                            "opcode": "EventSemaphore",
                            "outs": [],
                            "sync_info": {"on_update": [], "on_wait": [w]},
                        })
                    si["on_wait"] = [waits[-1]]
                out.append(inst)
            bb["instructions"] = out
    return ctr


class CompatBass(bass.Bass):
    def to_json_bytes(self) -> bytes:
        raw = super().to_json_bytes()
        bir = json.loads(raw)
        if _split_multi_waits(bir):
            return json.dumps(bir).encode()
        return raw


def _build_program():
    nc = CompatBass(num_devices=N_CORES)

    # packed inputs: fewer PJRT buffers -> lower per-call dispatch cost.
    # bpack (bf16): per layer [wqe|wqo|wke|wko|wv|wo] then the embed table.
    # fpack (f32):  [cosq|sinq|cosk|sink|trimask|normw].
    PLW = 8 * 128 * 1024            # per-layer bf16 weight block
    BP_EMB = 0                      # embed table first (gather needs offset 0)
    BP_W = T * D                    # weights after the table
    bpack_ext = nc.declare_dram_parameter("bpack", [L * PLW + T * D], BF16,
                                          isOutput=False)
    fpack_ext = nc.declare_dram_parameter("fpack", [4 * 128 * T + 128 * 128 + D],
                                          F32, isOutput=False)
    out_ext = nc.declare_dram_parameter("out", [256, D], F32, isOutput=True)

    def bview(off, shape):
        ap, s = [], 1
        for n in reversed(shape):
            ap.insert(0, [s, n])
            s *= n
        base = bpack_ext[:]
        return bass.AP(tensor=base.tensor, offset=off, ap=ap)

    def fview(off, shape):
        ap, s = [], 1
        for n in reversed(shape):
            ap.insert(0, [s, n])
            s *= n
        base = fpack_ext[:]
        return bass.AP(tensor=base.tensor, offset=off, ap=ap)

    WQE_O = BP_W
    WQO_O = BP_W + 128 * 1024
    WKE_O = BP_W + 2 * 128 * 1024
    WKO_O = BP_W + 3 * 128 * 1024
    WV_O = BP_W + 4 * 128 * 1024
    WO_O = BP_W + 4 * 128 * 1024 + 128 * 8 * 256
    emb_ext = bview(BP_EMB, [T, D])
    cosq_ext = fview(0, [128, T])
    sinq_ext = fview(128 * T, [128, T])
    cosk_ext = fview(2 * 128 * T, [128, T])
    sink_ext = fview(3 * 128 * T, [128, T])
    mask_ext = fview(4 * 128 * T, [128, 128])
    normw_off = 4 * 128 * T + 128 * 128

    from contextlib import ExitStack

    with tile.TileContext(nc) as tc, ExitStack() as stack:
        ec = stack.enter_context
        cpool = ec(tc.tile_pool(name="consts", bufs=1))
        xpool = ec(tc.tile_pool(name="xT", bufs=2))
        wpool = ec(tc.tile_pool(name="weights", bufs=2))
        qkpool = ec(tc.tile_pool(name="qk", bufs=1))
        rotpool = ec(tc.tile_pool(name="rot", bufs=2))
        tmppool = ec(tc.tile_pool(name="ropetmp", bufs=1))
        vpool = ec(tc.tile_pool(name="v", bufs=1))
        atpool = ec(tc.tile_pool(name="att", bufs=3))
        otpool = ec(tc.tile_pool(name="oT", bufs=2))
        xppool = ec(tc.tile_pool(name="xpart", bufs=1))
        gpool = ec(tc.tile_pool(name="gather", bufs=1))
        npool = ec(tc.tile_pool(name="norm", bufs=1))
        mpool = ec(tc.tile_pool(name="misc", bufs=1))
        dpool = ec(tc.tile_pool(name="dram", bufs=2, space="DRAM"))
        pspool = ec(tc.tile_pool(name="ps_mm", bufs=2, space="PSUM"))
        scpool = ec(tc.tile_pool(name="ps_sc", bufs=2, space="PSUM"))
        oaccpool = ec(tc.tile_pool(name="ps_oacc", bufs=1, space="PSUM"))

        identb = cpool.tile([128, 128], BF16, tag="identb")
        make_identity(nc, identb)
        trimask = cpool.tile([128, 128], F32, tag="trimask")
        nc.scalar.dma_start(out=trimask[:], in_=mask_ext)
        cosq = cpool.tile([128, T], F32, tag="cosq")
        nc.scalar.dma_start(out=cosq[:], in_=cosq_ext)
        sinq = cpool.tile([128, T], F32, tag="sinq")
        nc.scalar.dma_start(out=sinq[:], in_=sinq_ext)
        cosk = cpool.tile([128, T], F32, tag="cosk")
        nc.scalar.dma_start(out=cosk[:], in_=cosk_ext)
        sink = cpool.tile([128, T], F32, tag="sink")
        nc.scalar.dma_start(out=sink[:], in_=sink_ext)
        normw_b = cpool.tile([128, D], F32, tag="normw")
        _f = fpack_ext[:]
        nc.scalar.dma_start(
            out=normw_b[:],
            in_=bass.AP(tensor=_f.tensor, offset=normw_off,
                        ap=[[0, 128], [1, D]]),
        )
        ones64 = cpool.tile([1, 64], BF16, tag="ones64")
        nc.vector.memset(ones64[:], 1.0)
        epst = cpool.tile([128, 1], F32, tag="epst")
        nc.vector.memset(epst[:], EPS)

        def embed_half(h):
            """Load the host-pretransposed xT half tile (one DMA)."""
            xh = xpool.tile([128, 8, TH], BF16, tag=f"xh{h}", name=f"xh{h}")
            nc.sync.dma_start(
                out=xh[:], in_=bview(BP_EMB + h * 128 * 8 * TH, [128, 8, TH]))
            return xh

        xh = [embed_half(0), embed_half(1)]
        xh_next = [None, None]

        for l in range(L):
            # ---- weight loads (scalar queue; wpool bufs=2 prefetches)
            wqe = wpool.tile([128, 8, 128], BF16, tag="wqe")
            nc.scalar.dma_start(out=wqe[:], in_=bview(l * PLW + WQE_O, [128, 8, 128]))
            wqo = wpool.tile([128, 8, 128], BF16, tag="wqo")
            nc.scalar.dma_start(out=wqo[:], in_=bview(l * PLW + WQO_O, [128, 8, 128]))
            wke = wpool.tile([128, 8, 128], BF16, tag="wke")
            nc.scalar.dma_start(out=wke[:], in_=bview(l * PLW + WKE_O, [128, 8, 128]))
            wko = wpool.tile([128, 8, 128], BF16, tag="wko")
            nc.scalar.dma_start(out=wko[:], in_=bview(l * PLW + WKO_O, [128, 8, 128]))
            wv = wpool.tile([128, 8, 256], BF16, tag="wv")
            nc.scalar.dma_start(out=wv[:], in_=bview(l * PLW + WV_O, [128, 8, 256]))
            wo = []
            for g2 in range(2):
                wog = wpool.tile([128, D], BF16, tag=f"wo{g2}")
                nc.scalar.dma_start(
                    out=wog[:], in_=bview(l * PLW + WO_O + g2 * 128 * D, [128, D]))
                wo.append(wog)

            # rotated q/k tiles persist across both halves of the layer
            rot = {}
            for name in ("qe", "qo", "ke", "ko"):
                rot[name] = rotpool.tile([128, T], BF16, tag=f"r{name}",
                                         name=f"r{name}")
            r3 = {}
            for name in ("qe", "qo", "ke", "ko"):
                r3[name] = rotpool.tile([32, T], BF16, tag=f"r3{name}",
                                        name=f"r3{name}")
            vt = []
            for j in range(8):
                vj = vpool.tile([128, 4, 65], BF16, tag=f"v{j}", name=f"v{j}")
                vt.append(vj)
            oT = []
            for g2 in range(2):
                oTg = otpool.tile([128, T], BF16, tag=f"oT{g2}",
                                  name=f"oT{g2}")
                oT.append(oTg)
            hsl = []
            for h in range(4):
                if h < 3:
                    sl = slice(32 * h, 32 * h + 32)
                    hsl.append((rot["qe"][sl, :], rot["qo"][sl, :],
                                rot["ke"][sl, :], rot["ko"][sl, :]))
                else:
                    hsl.append((r3["qe"][:], r3["qo"][:],
                                r3["ke"][:], r3["ko"][:]))

            for half in range(2):
                hsl_ = ds(TH * half, TH)
                if l > 0:
                    xh[half] = xh_next[half]

                # ---- q/k projections for this half (even/odd components)
                comps = {}
                for name, wt in (("qe", wqe), ("qo", wqo),
                                 ("ke", wke), ("ko", wko)):
                    dst = qkpool.tile([128, TH], BF16, tag=f"c{name}")
                    ps = pspool.tile([128, TH], F32, tag="mm")
                    for k in range(8):
                        nc.tensor.matmul(
                            ps[:], lhsT=wt[:, k, :], rhs=xh[half][:, k, :],
                            start=(k == 0), stop=(k == 7),
                        )
                    nc.scalar.copy(out=dst[:], in_=ps[:])
                    comps[name] = dst

                # ---- RoPE for this half; q on DVE, k on Pool-free DVE mix
                for pre, cs, sn in (("q", cosq, sinq), ("k", cosk, sink)):
                    e_in, o_in = comps[pre + "e"], comps[pre + "o"]
                    ta = tmppool.tile([128, TH], F32, tag="tmpa")
                    tb = tmppool.tile([128, TH], F32, tag="tmpb")
                    nc.vector.tensor_mul(ta[:], e_in[:], cs[:, hsl_])
                    nc.vector.tensor_mul(tb[:], o_in[:], sn[:, hsl_])
                    nc.vector.tensor_tensor(
                        out=rot[pre + "e"][:, hsl_], in0=ta[:], in1=tb[:],
                        op=mybir.AluOpType.subtract)
                    tc2 = tmppool.tile([128, TH], F32, tag="tmpa")
                    td = tmppool.tile([128, TH], F32, tag="tmpb")
                    nc.vector.tensor_mul(tc2[:], e_in[:], sn[:, hsl_])
                    nc.vector.tensor_mul(td[:], o_in[:], cs[:, hsl_])
                    nc.vector.tensor_tensor(
                        out=rot[pre + "o"][:, hsl_], in0=tc2[:], in1=td[:],
                        op=mybir.AluOpType.add)

                # head 3 lives at partition base 96 (not addressable by the
                # PE); copy its 32 rows to base-0 tiles.
                for name in ("qe", "qo", "ke", "ko"):
                    nc.vector.tensor_copy(out=r3[name][:, hsl_],
                                          in_=rot[name][96:128, hsl_])

                # ---- v projection for this half, [t, c] layout, +ones col
                for jj in range(4):
                    j = 4 * half + jj
                    ps = pspool.tile([128, 256], F32, tag="mm")
                    for k in range(8):
                        nc.tensor.matmul(
                            ps[:], lhsT=xh[half][:, k, ts(jj, 128)],
                            rhs=wv[:, k, :],
                            start=(k == 0), stop=(k == 7),
                        )
                    nc.vector.memset(vt[j][:, :, 64:65], 1.0)
                    nc.vector.tensor_copy(
                        out=vt[j][:, :, 0:64],
                        in_=ps[:].rearrange("p (h d) -> p h d", h=4))

                # ---- attention for query half `half`
                qc = half
                oaccs = []
                for h in range(4):
                    oacch = oaccpool.tile([65, TH], F32, tag=f"oacc{h}",
                                          name=f"oacc{h}")
                    oaccs.append(oacch)
                jbmax = 3 if qc == 0 else 7
                for jb in range(jbmax + 1):
                    q_lo = max(TH * qc, 128 * jb)
                    n = TH * (qc + 1) - q_lo
                    ats = []
                    for h in range(4):
                        lqe, lqo, lke, lko = hsl[h]
                        sc = scpool.tile([128, TH], F32, tag="sc")
                        nc.tensor.matmul(
                            sc[:, :n], lhsT=lke[:, ts(jb, 128)],
                            rhs=lqe[:, ds(q_lo, n)], start=True, stop=False)
                        nc.tensor.matmul(
                            sc[:, :n], lhsT=lko[:, ts(jb, 128)],
                            rhs=lqo[:, ds(q_lo, n)], start=False, stop=True)
                        if q_lo == 128 * jb:
                            nc.vector.tensor_add(
                                out=sc[:, 0:128], in0=sc[:, 0:128],
                                in1=trimask[:])
                        at = atpool.tile([128, TH], BF16, tag="att")
                        nc.scalar.activation(out=at[:, :n], in_=sc[:, :n],
                                             func=Exp)
                        ats.append(at)
                    for h in range(4):
                        nc.tensor.matmul(
                            oaccs[h][:, ds(q_lo - TH * qc, n)],
                            lhsT=vt[jb][:, h, :], rhs=ats[h][:, :n],
                            start=(jb == 0), stop=(jb == jbmax))
                for h in range(4):
                    recip = mpool.tile([1, TH], F32, tag="recip")
                    nc.vector.reciprocal(out=recip[:], in_=oaccs[h][64:65, :])
                    recipb = mpool.tile([1, TH], BF16, tag="recipb")
                    nc.vector.tensor_copy(out=recipb[:], in_=recip[:])
                    bc_ps = scpool.tile([64, TH], F32, tag="sc")
                    nc.tensor.matmul(
                        bc_ps[:], lhsT=ones64[:], rhs=recipb[:],
                        start=True, stop=True)
                    bc_sb = mpool.tile([64, TH], F32, tag="bcsb")
                    nc.scalar.copy(out=bc_sb[:], in_=bc_ps[:])
                    if h % 2 == 0:
                        nc.vector.tensor_mul(
                            oT[h // 2][0:64, ds(TH * qc, TH)],
                            oaccs[h][0:64, :], bc_sb[:])
                    else:
                        # engine ops cannot write at a nonzero partition
                        # base on this toolchain; place via DMA instead
                        om = mpool.tile([64, TH], BF16, tag="om")
                        nc.vector.tensor_mul(om[:], oaccs[h][0:64, :],
                                             bc_sb[:])
                        nc.scalar.dma_start(
                            out=oT[h // 2][64:128, ds(TH * qc, TH)],
                            in_=om[:])

                # ---- o_proj partials for this half
                xpart = xppool.tile([128, 8, TH], BF16, tag=f"xp{half}",
                                    name=f"xp{half}")
                for e in range(8):
                    ps = pspool.tile([128, TH], F32, tag="mm")
                    for g2 in range(2):
                        nc.tensor.matmul(
                            ps[:], lhsT=wo[g2][:, ts(e, 128)],
                            rhs=oT[g2][:, ds(TH * qc, TH)],
                            start=(g2 == 0), stop=(g2 == 1))
                    nc.scalar.copy(out=xpart[:, e, :], in_=ps[:])

                if l < L - 1:
                    # ---- pipelined per-half all-reduce
                    arin = dpool.tile([128, 8, TH], BF16, tag=f"arin{half}")
                    arout = dpool.tile([128, 8, TH], BF16, tag=f"arout{half}")
                    nc.sync.dma_start(out=arin[:], in_=xpart[:])
                    nc.gpsimd.collective_compute(
                        "AllReduce", mybir.AluOpType.add,
                        ins=[arin[:]], outs=[arout[:]], replica_groups=GROUPS)
                    xh_next[half] = xpool.tile([128, 8, TH], BF16,
                                               tag=f"xh{half}",
                                               name=f"xh{half}n")
                    nc.sync.dma_start(out=xh_next[half], in_=arout[:])
                else:
                    # ---- last layer: reduce-scatter; rank r of each group
                    # receives the summed x for token chunk r.
                    if half == 0:
                        rsin = dpool.tile([4, 128, 8, 256], BF16, tag="rsin")
                        rsout = dpool.tile([128, 8, 256], BF16, tag="rsout")
                    for jch in range(2):
                        nc.sync.dma_start(
                            out=rsin[2 * half + jch],
                            in_=xpart[:, :, ds(256 * jch, 256)])
                    if half == 1:
                        nc.gpsimd.collective_compute(
                            "ReduceScatter", mybir.AluOpType.add,
                            ins=[rsin[:]], outs=[rsout[:]],
                            replica_groups=GROUPS)
                        xfin = xpool.tile([128, 8, 256], BF16, tag="xfin")
                        nc.sync.dma_start(out=xfin[:], in_=rsout[:])

        # ---- final RMSNorm (fp32) on this core's 256-token chunk
        for j in range(2):
            xrow = npool.tile([128, D], F32, tag="xrow")
            for k in range(8):
                tp = scpool.tile([128, 128], BF16, tag="sc")
                nc.tensor.transpose(tp[:], xfin[:, k, ts(j, 128)], identb[:])
                nc.vector.tensor_copy(out=xrow[:, ts(k, 128)], in_=tp[:])
            onorm = npool.tile([128, D], F32, tag="onorm")
            ssq = npool.tile([128, 1], F32, tag="ssq")
            nc.scalar.activation(out=onorm[:], in_=xrow[:], func=Square,
                                 accum_out=ssq[:])
            std = npool.tile([128, 1], F32, tag="std")
            nc.scalar.activation(out=std[:], in_=ssq[:], func=Sqrt,
                                 scale=1.0 / D, bias=epst[:, :1])
            rstd = npool.tile([128, 1], F32, tag="rstd")
            nc.vector.reciprocal(out=rstd[:], in_=std[:])
            nc.vector.tensor_scalar_mul(out=xrow[:], in0=xrow[:],
                                        scalar1=rstd[:, :1])
            nc.vector.tensor_mul(onorm[:], xrow[:], normw_b[:])
            nc.sync.dma_start(out=out_ext[ts(j, 128), :], in_=onorm[:])

    return nc


def _prep_inputs(toks, embed, Wq, Wk, Wv, Wo, norm_w):
    """Build the 8 per-core input maps from the full model inputs."""
    toks = np.asarray(toks)
    embed = np.asarray(embed, dtype=np.float32)
    Wq = np.asarray(Wq, dtype=np.float32)
    Wk = np.asarray(Wk, dtype=np.float32)
    Wv = np.asarray(Wv, dtype=np.float32)
    Wo = np.asarray(Wo, dtype=np.float32)
    norm_w = np.asarray(norm_w, dtype=np.float32)

    inv = 1.0 / (THETA ** (np.arange(0, HD, 2, dtype=np.float32) / HD))  # [32]
    ang = inv[:, None] * np.arange(T, dtype=np.float32)[None, :]         # [32, T]
    cos = np.cos(ang)
    sin = np.sin(ang)
    scale = 1.0 / math.sqrt(HD)
    cosq = np.tile(cos * scale, (4, 1)).astype(np.float32)
    sinq = np.tile(sin * scale, (4, 1)).astype(np.float32)
    cosk = np.tile(cos, (4, 1)).astype(np.float32)
    sink = np.tile(sin, (4, 1)).astype(np.float32)

    jj = np.arange(128)
    trimask = np.where(jj[:, None] <= jj[None, :], 0.0, -1e9).astype(np.float32)

    in_maps = []
    batch_tables = []
    for b in range(B):
        E = embed[np.asarray(toks[b], dtype=np.int64)].astype(
            ml_dtypes.bfloat16)                         # [T, D]
        xT = np.ascontiguousarray(
            E.reshape(2, T // 2, 8, 128).transpose(0, 3, 2, 1))
        batch_tables.append(xT)

    fpack = np.concatenate([cosq.ravel(), sinq.ravel(), cosk.ravel(),
                            sink.ravel(), trimask.ravel(), norm_w.ravel()])

    for c in range(N_CORES):
        b, g = c // 4, c % 4
        xT = batch_tables[b]
        heads = [4 * g + h for h in range(4)]
        ecols = np.concatenate([64 * ah + np.arange(0, 64, 2) for ah in heads])
        ocols = np.concatenate([64 * ah + np.arange(1, 64, 2) for ah in heads])
        vcols = np.arange(256 * g, 256 * g + 256)

        def tile_w(w):  # [L, D, 128 or 256] -> [L, 128, 8, n]
            n = w.shape[-1]
            return np.ascontiguousarray(
                w.reshape(L, 8, 128, n).transpose(0, 2, 1, 3)
            ).astype(ml_dtypes.bfloat16)

        wqe = tile_w(Wq[:, :, ecols]).reshape(L, -1)
        wqo = tile_w(Wq[:, :, ocols]).reshape(L, -1)
        wke = tile_w(Wk[:, :, ecols]).reshape(L, -1)
        wko = tile_w(Wk[:, :, ocols]).reshape(L, -1)
        wv = tile_w(Wv[:, :, vcols]).reshape(L, -1)
        wo = np.ascontiguousarray(
            Wo[:, vcols, :].reshape(L, 2, 128, D)).astype(
                ml_dtypes.bfloat16).reshape(L, -1)
        per_layer = np.concatenate([wqe, wqo, wke, wko, wv, wo], axis=1)
        bpack = np.concatenate([xT.ravel(), per_layer.ravel()])

        in_maps.append({
            "bpack": bpack,
            "fpack": fpack,
        })
    return in_maps


class _Runner:
    """Compile the SPMD program once; re-executable on the 8 cores.

    Mirrors concourse.bass2jax.run_bass_via_pjrt but caches an AOT
    fast-dispatch compile so repeated calls use the C++ dispatch path.
    """

    def __init__(self):
        import jax
        from jax.sharding import Mesh, NamedSharding, PartitionSpec

        try:
            from jax.experimental.shard_map import shard_map
        except ImportError:
            from jax.shard_map import shard_map

        from concourse import bass2jax

        bass2jax.install_neuronx_cc_hook()
        nc = _build_program()
        self._jax = jax

        partition_name = (
            nc.partition_id_tensor.name if nc.partition_id_tensor else None
        )
        in_names, out_names, out_avals, zero_outs = [], [], [], []
        for alloc in nc.m.functions[0].allocations:
            if not isinstance(alloc, mybir.MemoryLocationSet):
                continue
            name = alloc.memorylocations[0].name
            if alloc.kind == "ExternalInput":
                if name != partition_name:
                    in_names.append(name)
            elif alloc.kind == "ExternalOutput":
                out_names.append(name)
                shape = tuple(alloc.tensor_shape)
                dtype = mybir.dt.np(alloc.dtype)
                out_avals.append(jax.core.ShapedArray(shape, dtype))
                zero_outs.append(np.zeros(shape, dtype))
        self.in_names = list(in_names)
        self.out_names = out_names
        n_params = len(in_names)
        all_in_names = in_names + out_names
        if partition_name is not None:
            all_in_names = all_in_names + [partition_name]

        def _body(*args):
            operands = list(args)
            if partition_name is not None:
                operands.append(bass2jax.partition_id_tensor())
            outs = bass2jax._bass_exec_p.bind(
                *operands,
                out_avals=tuple(out_avals),
                in_names=tuple(all_in_names),
                out_names=tuple(out_names),
                lowering_input_output_aliases=(),
                sim_require_finite=True,
                sim_require_nnan=True,
                nc=nc,
            )
            return tuple(outs)

        devices = jax.devices()[:N_CORES]
        mesh = Mesh(np.asarray(devices), ("core",))
        in_specs = (PartitionSpec("core"),) * (n_params + len(out_names))
        out_specs = (PartitionSpec("core"),) * len(out_names)

        sh = NamedSharding(mesh, PartitionSpec("core"))
        in_structs = []
        for alloc in nc.m.functions[0].allocations:
            if not isinstance(alloc, mybir.MemoryLocationSet):
                continue
            name = alloc.memorylocations[0].name
            if alloc.kind == "ExternalInput" and name != partition_name:
                shape = tuple(alloc.tensor_shape)
                dtype = mybir.dt.np(alloc.dtype)
                in_structs.append(jax.ShapeDtypeStruct(
                    (N_CORES * shape[0], *shape[1:]), dtype, sharding=sh))
        for aval in out_avals:
            in_structs.append(jax.ShapeDtypeStruct(
                (N_CORES * aval.shape[0], *aval.shape[1:]), aval.dtype,
                sharding=sh))

        def _compile():
            return jax.jit(
                shard_map(_body, mesh=mesh, in_specs=in_specs,
                          out_specs=out_specs, check_rep=False),
                keep_unused=True,
            ).lower(*in_structs).compile()

        self._fn = bass2jax.fast_dispatch_compile(_compile)
        self._zero_outs = zero_outs
        self._out_avals = out_avals
        self._mesh = mesh
        self._pspec = PartitionSpec("core")

    def place(self, in_maps, on_device=False):
        cat = [
            np.concatenate([np.asarray(in_maps[c][n]) for c in range(N_CORES)],
                           axis=0)
            for n in self.in_names
        ]
        cat += [
            np.zeros((N_CORES * z.shape[0], *z.shape[1:]), z.dtype)
            for z in self._zero_outs
        ]
        if on_device:
            from jax.sharding import NamedSharding

            sh = NamedSharding(self._mesh, self._pspec)
            cat = [self._jax.device_put(a, sh) for a in cat]
        return cat

    def execute(self, placed):
        return self._fn(*placed)

    def run(self, in_maps):
        out_arrs = self.execute(self.place(in_maps))
        return [
            {
                n: np.asarray(out_arrs[i]).reshape(
                    N_CORES, *self._out_avals[i].shape)[c]
                for i, n in enumerate(self.out_names)
            }
            for c in range(N_CORES)
        ]


_CACHE = {}


def get_runner():
    if "runner" not in _CACHE:
        _CACHE["runner"] = _Runner()
    return _CACHE["runner"]


def kernel(toks, embed, Wq, Wk, Wv, Wo, norm_w):
    in_maps = _prep_inputs(toks, embed, Wq, Wk, Wv, Wo, norm_w)
    results = get_runner().run(in_maps)
    out = np.empty((B, T, D), dtype=np.float32)
    for c in range(N_CORES):
        b, g = c // 4, c % 4
        out[b, 256 * g: 256 * (g + 1)] = results[c]["out"]
    return out
